# revision 1
# baseline (speedup 1.0000x reference)
"""CPSF fused codebook kernel for Trainium2 (8 NeuronCores, codebook-parallel).

Sharding: M (codebook, 4096) split 8 ways -> 512 entries/core; every core sees
all B=2048 queries (large free dim amortizes per-instruction overhead). Host
sums the 8 partial [B,S] outputs.

Per (b,m,k):  Phi_k = ln(alpha w_k) + G*q_par_k + c_o*q_perp + c_o*dist_d
              wgt = sum_k exp(Phi_k);  T = wgt @ That
Factored:     base = sgn*|Gd|*(x^2+y^2) + F3   (F3: one PE-accumulated field,
              holds all q0/dist_d/cross/log terms + the umid*x range shift)
              E_k = exp(u'_k[m]*x + v'_k[m])   (ACT per-partition scale/bias)
              wgt = exp(base) * sum_k E_k
"""

import numpy as np

B, M, N, S, K = 2048, 4096, 64, 64, 8
EPS = 1e-3
NCORES = 8
ML = M // NCORES          # 512 codebook entries per core
MT = ML // 128            # 4 m-tiles per core
NQ = 4                    # b-quarters (PSUM-sized chunks of 512)
BQ = B // NQ              # 512
f32 = np.float32

_CACHE = {}
KSUM_BF16 = False


def _prep(z_re, z_im, d_re, d_im, zj_re, zj_im, dj_re, dj_im,
          That_re, That_im, alpha, sig_par, sig_perp):
    """Host-side packing: fp64 exact, cast to fp32 at the end."""
    x64 = lambda a: np.asarray(a, np.float64)
    zr, zi, dr, di = map(x64, (z_re, z_im, d_re, d_im))
    zjr, zji, djr, dji = map(x64, (zj_re, zj_im, dj_re, dj_im))

    tgl, wgl = np.polynomial.legendre.leggauss(K)
    t = (0.5 * (tgl + 1.0)).astype(f32).astype(np.float64)
    wq = (0.5 * wgl).astype(f32).astype(np.float64)

    dd2 = (djr**2 + dji**2).sum(-1)                          # [M]
    c_re = (djr * zjr + dji * zji).sum(-1)
    c_im = (djr * zji - dji * zjr).sum(-1)
    sp2 = x64(sig_par)**2 + EPS
    so2 = x64(sig_perp)**2 + EPS
    G = -0.5 / sp2
    c_o = -0.5 / so2
    Gd = G - c_o
    umid = -G * dd2
    lnal = np.log(np.maximum(x64(alpha), 1e-38))
    nzj = (zjr**2 + zji**2).sum(-1)
    nz = (zr**2 + zi**2).sum(-1)                             # [B]
    nd = (dr**2 + di**2).sum(-1)

    u = np.stack([-2.0 * G * t[k] * dd2 for k in range(K)])  # [K,M]
    up = u - umid[None, :]
    vp = np.stack([np.log(wq[k]) + G * (t[k] * dd2)**2 - up[k] * c_re
                   for k in range(K)])

    djx = np.concatenate([djr.T, dji.T], 0)                  # [128, M]
    djy = np.concatenate([-dji.T, djr.T], 0)
    f3z = ((-2.0 * c_o) * np.concatenate([zjr.T, zji.T], 0)
           + (-2.0 * Gd * c_re + umid) * djx
           + (-2.0 * Gd * c_im) * djy)
    f3d = (-2.0 * c_o) * np.concatenate([djr.T, dji.T], 0)
    const0 = (c_o * (nzj + dd2) + Gd * (c_re**2 + c_im**2)
              + lnal - umid * c_re)
    f3c = np.stack([c_o, c_o])                               # [2, M]
    rhsc = np.stack([nz, nd])                                # [2, B]
    that2 = np.concatenate([x64(That_re), x64(That_im)], 1)  # [M, 128]

    # pk: per m-tile block [djx | djy | f3z | f3d], each 128 cols
    nt = M // 128
    pk = np.empty((128, nt * 512), np.float64)
    for j in range(nt):
        ms = slice(j * 128, (j + 1) * 128)
        pk[:, j * 512 + 0:j * 512 + 128] = djx[:, ms]
        pk[:, j * 512 + 128:j * 512 + 256] = djy[:, ms]
        pk[:, j * 512 + 256:j * 512 + 384] = f3z[:, ms]
        pk[:, j * 512 + 384:j * 512 + 512] = f3d[:, ms]

    pconst = const0.reshape(nt, 128).T                       # [128, nt]
    psq = np.sqrt(np.abs(Gd)).reshape(nt, 128).T             # [128, nt]
    psgn = np.where(Gd >= 0, 1.0, -1.0).reshape(nt, 128).T
    pu = up.reshape(K, nt, 128).transpose(2, 0, 1).reshape(128, K * nt)
    pv = vp.reshape(K, nt, 128).transpose(2, 0, 1).reshape(128, K * nt)

    c = lambda a: np.ascontiguousarray(a, dtype=f32)
    return dict(pk=c(pk), f3c=c(f3c), that2=c(that2), psq=c(psq),
                psgn=c(psgn), pconst=c(pconst), pu=c(pu), pv=c(pv),
                zst=c(zst_ := np.concatenate([zr.T, zi.T], 0)),
                dst=c(np.concatenate([dr.T, di.T], 0)), rhsc=c(rhsc))


def _core_slices(p, cid):
    """Per-core in_map from the full packed arrays (m-sharded)."""
    jt = slice(cid * MT * 512, (cid + 1) * MT * 512)         # pk cols
    ms = slice(cid * ML, (cid + 1) * ML)
    jc = slice(cid * MT, (cid + 1) * MT)
    kc = np.concatenate([np.arange(k * (M // 128) + cid * MT,
                                   k * (M // 128) + (cid + 1) * MT)
                         for k in range(K)])
    cc = np.ascontiguousarray
    return {"pk": cc(p["pk"][:, jt]), "f3c": cc(p["f3c"][:, ms]),
            "that2": cc(p["that2"][ms, :]), "psq": cc(p["psq"][:, jc]),
            "psgn": cc(p["psgn"][:, jc]), "pconst": cc(p["pconst"][:, jc]),
            "pu": cc(p["pu"][:, kc]),
            "pv": cc(p["pv"][:, kc]), "zst": p["zst"], "dst": p["dst"],
            "rhsc": p["rhsc"]}


def _device_maps(maps):
    dev_maps = []
    for m in maps:
        dm = {k: m[k] for k in ("pk", "f3c", "that2", "zst", "dst", "rhsc")}
        dm["params"] = np.ascontiguousarray(np.concatenate(
            [m["psq"], m["psgn"], m["pconst"], m["pu"], m["pv"]], axis=1))
        dev_maps.append(dm)
    return dev_maps


def _emulate_core(m):
    """Numpy emulation of one core's device program."""
    zst, dst, rhsc = m["zst"], m["dst"], m["rhsc"]
    t_acc = np.zeros((128, 2048), f32)
    for j in range(MT):
        pkj = m["pk"][:, j * 512:(j + 1) * 512]
        djx_t, djy_t = pkj[:, 0:128], pkj[:, 128:256]
        f3z_t, f3d_t = pkj[:, 256:384], pkj[:, 384:512]
        x = (djx_t.T @ zst).astype(f32)
        y = (djy_t.T @ zst).astype(f32)
        F3 = (f3z_t.T @ zst + f3d_t.T @ dst
              + m["f3c"][:, j * 128:(j + 1) * 128].T @ rhsc
              + m["pconst"][:, j:j + 1]).astype(f32)
        sq = m["psq"][:, j:j + 1]
        xx = np.square(x * sq, dtype=f32)
        yy = np.square(y * sq, dtype=f32)
        Q = (xx + yy).astype(f32)
        Qs = (Q * m["psgn"][:, j:j + 1]).astype(f32)
        base = (F3 + Qs).astype(f32)
        Ssum = np.zeros_like(x)
        for k in range(K):
            col = k * MT + j
            arg = (x * m["pu"][:, col:col + 1] + m["pv"][:, col:col + 1]).astype(f32)
            Ssum = (Ssum + np.exp(arg, dtype=f32)).astype(f32)
        wgt = (np.exp(base, dtype=f32) * Ssum).astype(f32)
        that_t = m["that2"][j * 128:(j + 1) * 128, :]
        t_acc += (that_t.T @ wgt).astype(f32)
    return t_acc


def _build_bass():
    import concourse.bacc as bacc
    import concourse.mybir as mybir
    from concourse import tile

    dt = mybir.dt.float32
    AF = mybir.ActivationFunctionType
    AO = mybir.AluOpType
    nc = bacc.Bacc("TRN2", target_bir_lowering=False, debug=False)

    bfdt = mybir.dt.bfloat16
    dram = {}
    for name, shape, dty in [("zst", [128, B], dt), ("dst", [128, B], dt),
                             ("rhsc", [2, B], dt),
                             ("pk", [128, MT * 512], dt),
                             ("f3c", [2, ML], dt), ("that2", [ML, 128], dt),
                             ("params", [128, 3 * MT + 2 * K * MT], dt)]:
        dram[name] = nc.dram_tensor(name, shape, dty, kind="ExternalInput")
    tout = nc.dram_tensor("tout", [128, B], dt, kind="ExternalOutput")

    HB = B // 2
    edt = mybir.dt.bfloat16 if KSUM_BF16 else dt
    with tile.TileContext(nc) as tc:
        with tc.tile_pool(name="const", bufs=1) as cpool:
            params = cpool.tile([128, 3 * MT + 2 * K * MT], dt)
            psq = params[:, 0:MT]
            psgn = params[:, MT:2 * MT]
            pconst = params[:, 2 * MT:3 * MT]
            pu = params[:, 3 * MT:3 * MT + K * MT]
            pv = params[:, 3 * MT + K * MT:3 * MT + 2 * K * MT]
            rhsc = cpool.tile([2, B], dt)
            that_all = cpool.tile([128, MT * 128], dt)
            f3c_all = cpool.tile([2, ML], dt)
            warm = cpool.tile([128, 8], dt)
            zqs = [cpool.tile([128, BQ], dt, name=f"zq{i}") for i in range(NQ)]
            dqs = [cpool.tile([128, BQ], dt, name=f"dq{i}") for i in range(NQ)]
            nc.sync.dma_start(params[:, :], dram["params"][:, :])
            # fire the ACT exp table-load ASAP, overlapping remaining DMAs
            nc.scalar.activation(warm[:, :], params[:, 0:8], AF.Exp)


            with (
                tc.tile_pool(name="lhs", bufs=2) as lpool,
                tc.tile_pool(name="work", bufs=1) as wpool,
                tc.tile_pool(name="eslab", bufs=1) as epool,
                tc.tile_pool(name="wgtp", bufs=1) as gpool,
            ):
                # DMA order = sync emission order: critical-path first
                pk0x = lpool.tile([128, 128], dt, tag="pkx")
                pk0 = lpool.tile([128, 512], dt, tag="pk")
                nc.sync.dma_start(zqs[0][:, :], dram["zst"][:, 0:BQ])
                nc.sync.dma_start(pk0x[:, :], dram["pk"][:, 0:128])
                nc.sync.dma_start(zqs[1][:, :], dram["zst"][:, BQ:2 * BQ])
                nc.sync.dma_start(zqs[2][:, :], dram["zst"][:, 2 * BQ:3 * BQ])
                nc.sync.dma_start(zqs[3][:, :], dram["zst"][:, 3 * BQ:4 * BQ])
                nc.sync.dma_start(pk0[:, :], dram["pk"][:, 0:512])
                for i in range(NQ):
                    qs = slice(i * BQ, (i + 1) * BQ)
                    nc.sync.dma_start(dqs[i][:, :], dram["dst"][:, qs])
                for t_, d_ in [(rhsc, "rhsc"), (f3c_all, "f3c")]:
                    nc.sync.dma_start(t_[:, :], dram[d_][:, :])
                nc.sync.dma_start(
                    that_all[:, :].rearrange("p (j c) -> p j c", j=MT),
                    dram["that2"][:, :].rearrange("(j p) c -> p j c", p=128))
                wgts = []
                with (
                    tc.tile_pool(name="xps", bufs=1, space="PSUM") as xpool,
                    tc.tile_pool(name="yf", bufs=2, space="PSUM") as qpool,
                    tc.tile_pool(name="tp0", bufs=1, space="PSUM") as tpool0,
                ):
                    tph0 = tpool0.tile([128, HB], dt, tag="tph0")
                    for j in range(MT):
                        if j == 0:
                            pk_t, pkx_t = pk0, pk0x
                        else:
                            pk_t = lpool.tile([128, 512], dt, tag="pk")
                            pkx_t = lpool.tile([128, 128], dt, tag="pkx")
                            nc.sync.dma_start(
                                pkx_t[:, :],
                                dram["pk"][:, j * 512:j * 512 + 128])
                            nc.sync.dma_start(
                                pk_t[:, :],
                                dram["pk"][:, j * 512:(j + 1) * 512])
                        djx_t = pkx_t[:, :]
                        djy_t = pk_t[:, 128:256]
                        f3z_t = pk_t[:, 256:384]
                        f3d_t = pk_t[:, 384:512]
                        f3c_t = f3c_all[:, j * 128:(j + 1) * 128]

                        xx = wpool.tile([128, B], dt, tag="xx")
                        yy = wpool.tile([128, B], dt, tag="yy")
                        EB = wpool.tile([128, B], dt, tag="EB")
                        slabs = [epool.tile([128, 2 * B], edt, tag=f"esl{p}",
                                            name=f"esl{p}_{j}")
                                 for p in range(4)]
                        wgt = gpool.tile([128, B], dt, tag=f"wgt{j}")

                        x_ps = xpool.tile([128, B], dt, tag="x")
                        for q in range(NQ):
                            nc.tensor.matmul(x_ps[:, q * BQ:(q + 1) * BQ],
                                             djx_t, zqs[q][:, :],
                                             start=True, stop=True)
                        if j > 0:
                            that_p = that_all[:, (j - 1) * 128:j * 128]
                            for q2 in range(2):
                                qs2 = slice(q2 * BQ, (q2 + 1) * BQ)
                                nc.tensor.matmul(tph0[:, qs2], that_p,
                                                 wgts[j - 1][:, qs2],
                                                 start=(j - 1 == 0),
                                                 stop=False)

                        def E(k):
                            col = k * MT + j
                            nc.scalar.activation(
                                slabs[k // 2][:, (k % 2) * B:(k % 2 + 1) * B],
                                x_ps[:, :], AF.Exp,
                                bias=pv[:, col:col + 1],
                                scale=pu[:, col:col + 1])

                        yfs = []

                        def YFpe(q):
                            qs = slice(q * BQ, (q + 1) * BQ)
                            yf = qpool.tile([128, BQ], dt, tag="yf",
                                            name=f"yf{j}_{q}")
                            nc.tensor.matmul(yf[:, :], djy_t, zqs[q][:, :],
                                             start=True, stop=True)
                            yfs.append((yf, qs))

                        def YFsq(q):
                            yf, qs = yfs[q]
                            nc.scalar.activation(yy[:, qs], yf[:, :],
                                                 AF.Square,
                                                 scale=psq[:, j:j + 1])
                            nc.tensor.matmul(yf[:, :], f3z_t, zqs[q][:, :],
                                             start=True, stop=False)
                            nc.tensor.matmul(yf[:, :], f3d_t, dqs[q][:, :],
                                             start=False, stop=False)
                            nc.tensor.matmul(yf[:, :], f3c_t, rhsc[:, qs],
                                             start=False, stop=True)

                        def BASEq(q):
                            yf, qs = yfs[q]
                            nc.vector.tensor_add(xx[:, qs], xx[:, qs],
                                                 yy[:, qs])
                            nc.vector.tensor_scalar(xx[:, qs], xx[:, qs],
                                                    psgn[:, j:j + 1],
                                                    pconst[:, j:j + 1],
                                                    AO.mult, AO.add)
                            nc.vector.tensor_add(xx[:, qs], xx[:, qs],
                                                 yf[:, :])

                        sa = wpool.tile([128, B], edt, tag="sa")
                        sb = wpool.tile([128, B], edt, tag="sb")
                        sc = wpool.tile([128, B], edt, tag="sc")
                        SL = lambda p, i: slabs[p][:, i * B:(i + 1) * B]

                        def YF(q):
                            YFsq(q)
                            BASEq(q)

                        for q in range(NQ):
                            YFpe(q)
                        E(0); E(1)
                        nc.scalar.activation(xx[:, :], x_ps[:, :], AF.Square,
                                             scale=psq[:, j:j + 1])
                        YF(0)
                        E(2); E(3)
                        YF(1)
                        E(4)
                        YF(2)
                        E(5); E(6)
                        YF(3)
                        E(7)
                        nc.vector.tensor_add(sa[:, :], SL(0, 0), SL(0, 1))
                        nc.vector.tensor_add(sb[:, :], SL(1, 0), SL(1, 1))
                        nc.vector.tensor_add(sa[:, :], sa[:, :], sb[:, :])
                        nc.vector.tensor_add(sb[:, :], SL(2, 0), SL(2, 1))
                        nc.vector.tensor_add(sc[:, :], SL(3, 0), SL(3, 1))
                        nc.vector.tensor_add(sb[:, :], sb[:, :], sc[:, :])
                        nc.vector.tensor_add(sa[:, :], sa[:, :], sb[:, :])
                        nc.scalar.activation(EB[:, :], xx[:, :], AF.Exp)
                        nc.vector.tensor_mul(wgt[:, :], EB[:, :], sa[:, :])
                        wgts.append(wgt)

                    ocp = wpool.tile([128, B], dt, tag="xx")
                    that_l = that_all[:, (MT - 1) * 128:MT * 128]
                    for q2 in range(2):
                        qs2 = slice(q2 * BQ, (q2 + 1) * BQ)
                        nc.tensor.matmul(tph0[:, qs2], that_l,
                                         wgts[MT - 1][:, qs2],
                                         start=False, stop=True)
                        nc.vector.tensor_copy(ocp[:, qs2], tph0[:, qs2])
                        nc.sync.dma_start(tout[:, qs2], ocp[:, qs2])

                with tc.tile_pool(name="tp1", bufs=1, space="PSUM") as tpool1:
                    tph1 = tpool1.tile([128, HB], dt, tag="tph1")
                    for q2 in range(2):
                        qg = 2 + q2
                        qs = slice(qg * BQ, (qg + 1) * BQ)
                        qs2 = slice(q2 * BQ, (q2 + 1) * BQ)
                        for j in range(MT):
                            that_j = that_all[:, j * 128:(j + 1) * 128]
                            nc.tensor.matmul(tph1[:, qs2], that_j,
                                             wgts[j][:, qs],
                                             start=(j == 0),
                                             stop=(j == MT - 1))
                        nc.vector.tensor_copy(ocp[:, qs], tph1[:, qs2])
                        nc.sync.dma_start(tout[:, qs], ocp[:, qs])

    nc.compile()
    return nc


def kernel(z_re, z_im, d_re, d_im, zj_re, zj_im, dj_re, dj_im,
           That_re, That_im, alpha, sig_par, sig_perp, _emulate=False):
    p = _prep(z_re, z_im, d_re, d_im, zj_re, zj_im, dj_re, dj_im,
              That_re, That_im, alpha, sig_par, sig_perp)
    maps = [_core_slices(p, c) for c in range(NCORES)]

    if _emulate:
        outs = [_emulate_core(m) for m in maps]
    else:
        from concourse.bass_utils import run_bass_kernel_spmd
        if "nc" not in _CACHE:
            _CACHE["nc"] = _build_bass()
        dev_maps = _device_maps(maps)
        res = run_bass_kernel_spmd(_CACHE["nc"], dev_maps,
                                   core_ids=list(range(NCORES)))
        outs = [res.results[c]["tout"] for c in range(NCORES)]

    full = np.zeros((128, B), np.float64)
    for o in outs:
        full += o.astype(np.float64)
    full = full.astype(f32).T                   # [B, 128]
    return (full[:, :S] + 1j * full[:, S:]).astype(np.complex64)

